# revision 1
# baseline (speedup 1.0000x reference)
"""Llama decoder block on 8 trn2 NeuronCores.

Sharding: DP2 x TP4 (core c -> batch c//4, group g=c%4 of 4 heads / 1024 d_ff
columns). One on-device AllReduce (groups [[0-3],[4-7]]) after out_proj; the
post-down-proj reduction is done on the host (partials summed per batch).

Device dataflow (per core, everything feature-major so all linear matmuls
contract on the partition dim with zero on-device transposes):
  h.T   = x.T * rstd1 (rstd1 host-precomputed; norm1_w folded into weights)
  qk.T  = Wqk.T-as-lhsT @ h.T   (cols host-permuted to [q_lo|q_hi|k_lo|k_hi])
  v     = h.T-as-lhsT @ Wv      (token-major, augmented with a ones column)
  RoPE on qk.T via host cos/sin tiles, repack per head via SBUF-SBUF DMA
  scores.T = k_h-as-lhsT @ q_h  ([tk,tq]; exp with scale=1/8, no max needed)
  ctx.T | l = expT-as-lhsT? no: lhsT=v_aug, rhs=expT -> [65, tq] psum accum
  y.T partial = Wo-as-lhsT @ ctx.T -> AllReduce
  y2.T  = AR + x.T + bo;  h2.T = y2.T * rstd2 (norm2_w folded into Wg/Wu)
  act.T = silu(Wg.T@h2.T) * (Wu.T@h2.T);  z.T = Wd.T@act.T
  out_partial.T = z.T + 0.25*y2.T  (host sums 4 partials per batch)
"""

import numpy as np
from contextlib import ExitStack

import concourse.bass as bass
import concourse.tile as tile
from concourse import bacc, mybir
from concourse.bass_utils import run_bass_kernel_spmd

# model dims (hardcoded per problem spec)
D = 1024
H = 16
HD = 64
DFF = 4096
B = 2
T = 2048
EPS = 1e-6
ROPE_BASE = 10000.0

NCORES = 8
TPG = 4              # tensor-parallel group size
HG = H // TPG        # 4 heads per core
QKW = HG * HD * 2    # 512 qk cols per core
VW = HG * HD         # 256 v cols per core
FFS = DFF // TPG     # 1024 ff cols per core
P = 128
KS = D // P          # 8 contraction subtiles for d_model
NTQ = 4
TQ = T // NTQ        # 512-token chunks
NTOK = T // P        # 16 token tiles of 128
FP = mybir.dt.float32

_CACHE = {}


def _build_nc():
    nc = bacc.Bacc("TRN2", target_bir_lowering=False, num_devices=NCORES)

    dt_in = {}
    def din(name, shape):
        dt_in[name] = nc.dram_tensor(name, list(shape), FP, kind="ExternalInput")
        return dt_in[name]

    xT = din("xT", (D, T))            # x[b].T
    rstd1 = din("rstd1", (1, T))
    cosT = din("cosT", (P, T))        # [4 heads x 32 pairs, T]
    sinT = din("sinT", (P, T))
    wqk = din("wqk", (D, QKW))        # cols: [q_lo(128) | q_hi(128) | k_lo | k_hi]
    bqk = din("bqk", (QKW,))
    wv = din("wv", (D, VW))
    bv = din("bv", (1, VW))
    wo = din("wo", (VW, D))           # rows = this core's ctx features
    bo = din("bo", (D,))
    wg = din("wg", (D, FFS))
    wu = din("wu", (D, FFS))
    wd = din("wd", (FFS, D))

    outT = nc.dram_tensor("outT", [D, T], FP, kind="ExternalOutput")

    ar_in = nc.dram_tensor("ar_in", [D, T], FP)
    ar_out = nc.dram_tensor("ar_out", [D, T], FP)

    with tile.TileContext(nc) as tc:
        _body(tc, xT, rstd1, cosT, sinT, wqk, bqk, wv, bv, wo, bo,
              wg, wu, wd, outT, ar_in, ar_out)
    nc.compile()
    return nc


def _body(tc, xT, rstd1, cosT, sinT, wqk, bqk, wv, bv, wo, bo,
          wg, wu, wd, outT, ar_in, ar_out):
    nc = tc.nc
    AF = mybir.ActivationFunctionType
    OP = mybir.AluOpType

    with ExitStack() as ctx:
        singles = ctx.enter_context(tc.tile_pool(name="singles", bufs=1))

        # ---- small persistent loads ----
        bqk_sb = singles.tile([P, QKW // P], FP)   # [p, tile] per-partition scalars
        nc.sync.dma_start(out=bqk_sb[:], in_=bqk.ap().rearrange("(i p) -> p i", p=P))
        bv_sb = singles.tile([P, VW], FP)
        nc.gpsimd.dma_start(out=bv_sb[:], in_=bv.ap().to_broadcast((P, VW)))
        bo_sb = singles.tile([P, KS], FP)
        nc.sync.dma_start(out=bo_sb[:], in_=bo.ap().rearrange("(i p) -> p i", p=P))
        ones_sb = singles.tile([P, 1], FP)
        nc.vector.memset(ones_sb[:], 1.0)
        eps_sb = singles.tile([1, 1], FP)
        nc.vector.memset(eps_sb[:], EPS)

        with ExitStack() as attn_ctx:
            _attn_phase(tc, attn_ctx, singles, xT, rstd1, cosT, sinT,
                        bqk_sb, bv_sb, wqk, wv, wo, ar_in)

        nc.gpsimd.collective_compute(
            "AllReduce", OP.add,
            replica_groups=[[0, 1, 2, 3], [4, 5, 6, 7]],
            ins=[ar_in.ap()], outs=[ar_out.ap()],
        )

        with ExitStack() as mlp_ctx:
            _mlp_phase(tc, mlp_ctx, xT, ar_out, bo_sb, ones_sb, eps_sb,
                       wg, wu, wd, outT)


def _attn_phase(tc, ctx, singles, xT, rstd1, cosT, sinT, bqk_sb, bv_sb,
                wqk, wv, wo, ar_in):
    nc = tc.nc
    AF = mybir.ActivationFunctionType
    OP = mybir.AluOpType

    wpool = ctx.enter_context(tc.tile_pool(name="attn_w", bufs=1))
    rstd1_sb = wpool.tile([P, T], FP)
    nc.gpsimd.dma_start(out=rstd1_sb[:], in_=rstd1.ap().to_broadcast((P, T)))
    cos_sb = wpool.tile([P, T], FP)
    nc.sync.dma_start(out=cos_sb[:], in_=cosT.ap())
    sin_sb = wpool.tile([P, T], FP)
    nc.sync.dma_start(out=sin_sb[:], in_=sinT.ap())
    wqk_sb = wpool.tile([P, KS, QKW], FP)
    nc.sync.dma_start(out=wqk_sb[:], in_=wqk.ap().rearrange("(ks p) m -> p ks m", p=P))
    wv_sb = wpool.tile([P, KS, VW], FP)
    nc.sync.dma_start(out=wv_sb[:], in_=wv.ap().rearrange("(ks p) m -> p ks m", p=P))
    wo_sb = wpool.tile([P, VW // P, D], FP)
    nc.sync.dma_start(out=wo_sb[:], in_=wo.ap().rearrange("(ks p) m -> p ks m", p=P))

    persist = ctx.enter_context(tc.tile_pool(name="attn_persist", bufs=1))
    # v storage token-major, per head slot of 66 cols: [v(64) | 1.0 | pad]
    vt = persist.tile([P, NTOK, HG, 66], FP)
    nc.vector.memset(vt[:, :, :, 64:65], 1.0)
    # rope'd per-head q/k: 2 tiles each holding 2 heads at partition 64*h
    qh = [persist.tile([P, T], FP, name=f"qh{i}") for i in range(2)]
    kh = [persist.tile([P, T], FP, name=f"kh{i}") for i in range(2)]
    # ctx.T accumulated [256, T] as [128, 2, T]
    ctxT = persist.tile([P, 2, T], FP)

    xpool = ctx.enter_context(tc.tile_pool(name="attn_x", bufs=2))
    qkpool = ctx.enter_context(tc.tile_pool(name="attn_qk", bufs=2))
    tpool = ctx.enter_context(tc.tile_pool(name="attn_tmp", bufs=2))
    epool = ctx.enter_context(tc.tile_pool(name="attn_exp", bufs=4))
    opool = ctx.enter_context(tc.tile_pool(name="attn_out", bufs=3))
    small = ctx.enter_context(tc.tile_pool(name="attn_small", bufs=2))

    psA = ctx.enter_context(tc.tile_pool(name="psA", bufs=2, space="PSUM"))
    psB = ctx.enter_context(tc.tile_pool(name="psB", bufs=2, space="PSUM"))
    psC = ctx.enter_context(tc.tile_pool(name="psC", bufs=2, space="PSUM"))

    # ---- qkv + rope + repack, chunked over 512 tokens ----
    for c in range(NTQ):
        cs = slice(c * TQ, (c + 1) * TQ)
        ht = xpool.tile([P, KS, TQ], FP, tag="ht")
        nc.sync.dma_start(
            out=ht[:],
            in_=xT.ap().rearrange("(ks p) t -> p ks t", p=P)[:, :, cs],
        )
        # h.T = x.T * rstd1 (broadcast along partitions), in place
        for ks in range(KS):
            nc.vector.tensor_tensor(
                ht[:, ks, :], ht[:, ks, :], rstd1_sb[:, cs], OP.mult,
            )

        # qk.T chunk: 4 psum tiles of [128, 512]
        qkc = qkpool.tile([P, 4, TQ], FP, tag="qkc")  # [qlo,qhi,klo,khi] chunk
        for m in range(4):
            ps = psB.tile([P, TQ], FP, tag="mm")
            for ks in range(KS):
                nc.tensor.matmul(ps[:], wqk_sb[:, ks, m * P:(m + 1) * P], ht[:, ks, :],
                                 start=(ks == 0), stop=(ks == KS - 1))
            # copy psum -> sbuf with bias add (per-partition scalar)
            nc.vector.tensor_scalar(
                out=qkc[:, m, :], in0=ps[:], scalar1=bqk_sb[:, m:m + 1], scalar2=None,
                op0=OP.add,
            )

        # v chunk: token-major
        for jj in range(TQ // P):
            j = c * (TQ // P) + jj
            psv_full = psB.tile([P, TQ], FP, tag="mm", name="psv")
            psv = psv_full[:, :VW]
            for ks in range(KS):
                nc.tensor.matmul(psv[:], ht[:, ks, jj * P:(jj + 1) * P], wv_sb[:, ks, :],
                                 start=(ks == 0), stop=(ks == KS - 1))
            nc.vector.tensor_tensor(
                vt[:, j, :, 0:64],
                psv.rearrange("p (h d) -> p h d", h=HG),
                bv_sb.rearrange("p (h d) -> p h d", h=HG),
                OP.add,
            )

        # rope on qk chunk: out_lo = lo*cos - hi*sin ; out_hi = lo*sin + hi*cos
        cs_cos = cos_sb[:, cs]
        cs_sin = sin_sb[:, cs]
        for pair in range(2):  # 0 = q, 1 = k
            lo = qkc[:, 2 * pair, :]
            hi = qkc[:, 2 * pair + 1, :]
            t_lo = tpool.tile([P, TQ], FP, tag="t_lo")
            t_hi = tpool.tile([P, TQ], FP, tag="t_hi")
            t3 = tpool.tile([P, TQ], FP, tag="t3")
            nc.vector.tensor_tensor(t_lo[:], lo, cs_cos, OP.mult)
            nc.vector.tensor_tensor(t3[:], hi, cs_sin, OP.mult)
            nc.vector.tensor_tensor(t_lo[:], t_lo[:], t3[:], OP.subtract)
            nc.vector.tensor_tensor(t_hi[:], lo, cs_sin, OP.mult)
            nc.vector.tensor_tensor(t3[:], hi, cs_cos, OP.mult)
            nc.vector.tensor_tensor(t_hi[:], t_hi[:], t3[:], OP.add)
            # repack: head h -> (tile h//2, partitions 64*(h%2) + [0:32 lo | 32:64 hi])
            dst = qh if pair == 0 else kh
            for h in range(HG):
                po = 64 * (h % 2)
                nc.sync.dma_start(out=dst[h // 2][po:po + 32, cs], in_=t_lo[32 * h:32 * h + 32, :])
                nc.sync.dma_start(out=dst[h // 2][po + 32:po + 64, cs], in_=t_hi[32 * h:32 * h + 32, :])

    # ---- attention: scores.T -> exp -> mask -> pv ----
    for c in range(NTQ):
        qs = slice(c * TQ, (c + 1) * TQ)
        for h in range(HG):
            po = 64 * (h % 2)
            qtile = qh[h // 2]
            ktile = kh[h // 2]
            pctx = psC.tile([65, TQ], FP, tag="pctx")
            nblk = 4 * c + 4
            for blk in range(nblk):
                pscore = psA.tile([P, TQ], FP, tag="pscore")
                nc.tensor.matmul(
                    pscore[:],
                    ktile[po:po + 64, blk * P:(blk + 1) * P],
                    qtile[po:po + 64, qs],
                    start=True, stop=True,
                )
                et = epool.tile([P, TQ], FP, tag="et")
                nc.scalar.activation(out=et[:], in_=pscore[:], func=AF.Exp, scale=0.125)
                r = blk - 4 * c
                if r >= 0:
                    # keep iff tq >= tk: j - p - 128r >= 0 else 0
                    nc.gpsimd.affine_select(
                        out=et[:], in_=et[:], compare_op=OP.is_ge, fill=0.0,
                        base=-P * r, channel_multiplier=-1, pattern=[[1, TQ]],
                    )
                nc.tensor.matmul(pctx[:], vt[:, blk, h, 0:65], et[:],
                                 start=(blk == 0), stop=(blk == nblk - 1))
            rl = small.tile([1, TQ], FP, tag="rl")
            nc.vector.reciprocal(rl[:], pctx[64:65, :])
            rlb = small.tile([64, TQ], FP, tag="rlb")
            nc.gpsimd.partition_broadcast(rlb[:], rl[0:1, :])
            nc.vector.tensor_tensor(
                ctxT[po:po + 64, h // 2, qs], pctx[0:64, :], rlb[:], OP.mult,
            )

    # ---- out_proj partial -> ar_in ----
    ar_in_r = ar_in.ap().rearrange("(m p) t -> p m t", p=P)
    for m in range(KS):
        for c in range(NTQ):
            pso = psB.tile([P, TQ], FP, tag="pso")
            for k2 in range(VW // P):
                nc.tensor.matmul(pso[:], wo_sb[:, k2, m * P:(m + 1) * P],
                                 ctxT[:, k2, c * TQ:(c + 1) * TQ],
                                 start=(k2 == 0), stop=(k2 == VW // P - 1))
            yo = opool.tile([P, TQ], FP, tag="yo")
            nc.vector.tensor_copy(out=yo[:], in_=pso[:])
            nc.sync.dma_start(out=ar_in_r[:, m, c * TQ:(c + 1) * TQ], in_=yo[:])


def _mlp_phase(tc, ctx, xT, ar_out, bo_sb, ones_sb, eps_sb, wg, wu, wd, outT):
    nc = tc.nc
    AF = mybir.ActivationFunctionType
    OP = mybir.AluOpType

    wpool = ctx.enter_context(tc.tile_pool(name="mlp_w", bufs=1))
    wg_sb = wpool.tile([P, KS, FFS], FP)
    nc.sync.dma_start(out=wg_sb[:], in_=wg.ap().rearrange("(ks p) m -> p ks m", p=P))
    wu_sb = wpool.tile([P, KS, FFS], FP)
    nc.sync.dma_start(out=wu_sb[:], in_=wu.ap().rearrange("(ks p) m -> p ks m", p=P))
    wd_sb = wpool.tile([P, FFS // P, D], FP)
    nc.sync.dma_start(out=wd_sb[:], in_=wd.ap().rearrange("(ks p) m -> p ks m", p=P))

    cpool = ctx.enter_context(tc.tile_pool(name="mlp_c", bufs=1))
    scpool = ctx.enter_context(tc.tile_pool(name="mlp_scratch", bufs=2))
    spool = ctx.enter_context(tc.tile_pool(name="mlp_s", bufs=2))
    psA = ctx.enter_context(tc.tile_pool(name="psMA", bufs=2, space="PSUM"))
    psB = ctx.enter_context(tc.tile_pool(name="psMB", bufs=2, space="PSUM"))
    psS = ctx.enter_context(tc.tile_pool(name="psMS", bufs=1, space="PSUM"))

    xT_r = xT.ap().rearrange("(ks p) t -> p ks t", p=P)
    ar_r = ar_out.ap().rearrange("(ks p) t -> p ks t", p=P)
    out_r = outT.ap().rearrange("(m p) t -> p m t", p=P)

    for c in range(NTQ):
        cs = slice(c * TQ, (c + 1) * TQ)
        y2 = cpool.tile([P, KS, TQ], FP, tag="y2")
        xc = scpool.tile([P, KS, TQ], FP, tag="scratch", name="xc")
        nc.sync.dma_start(out=y2[:], in_=ar_r[:, :, cs])
        nc.sync.dma_start(out=xc[:], in_=xT_r[:, :, cs])
        nc.vector.tensor_tensor(y2[:], y2[:], xc[:], OP.add)
        for ks in range(KS):
            nc.vector.tensor_scalar(
                out=y2[:, ks, :], in0=y2[:, ks, :],
                scalar1=bo_sb[:, ks:ks + 1], scalar2=None, op0=OP.add)

        # rmsnorm2: sumsq over features (partitions) via ones-matmul
        sq = scpool.tile([P, KS, TQ], FP, tag="scratch", name="sq")
        nc.vector.tensor_tensor(sq[:], y2[:], y2[:], OP.mult)
        pss = psS.tile([1, TQ], FP, tag="pss")
        for ks in range(KS):
            nc.tensor.matmul(pss[:], ones_sb[:], sq[:, ks, :],
                             start=(ks == 0), stop=(ks == KS - 1))
        rstd2 = spool.tile([1, TQ], FP, tag="rstd2")
        nc.scalar.activation(out=rstd2[:], in_=pss[:], func=AF.Sqrt,
                             bias=eps_sb[0:1, 0:1], scale=1.0 / D)
        nc.vector.reciprocal(rstd2[:], rstd2[:])
        rstd2_b = spool.tile([P, TQ], FP, tag="rstd2_b")
        nc.gpsimd.partition_broadcast(rstd2_b[:], rstd2[0:1, :])

        h2 = cpool.tile([P, KS, TQ], FP, tag="h2")
        for ks in range(KS):
            nc.vector.tensor_tensor(
                h2[:, ks, :], y2[:, ks, :], rstd2_b[:], OP.mult)

        # gate/up -> act (feature-major over ff shard)
        act = cpool.tile([P, FFS // P, TQ], FP, tag="act")
        for m in range(FFS // P):
            psg = psA.tile([P, TQ], FP, tag="psg")
            for ks in range(KS):
                nc.tensor.matmul(psg[:], wg_sb[:, ks, m * P:(m + 1) * P], h2[:, ks, :],
                                 start=(ks == 0), stop=(ks == KS - 1))
            psu = psB.tile([P, TQ], FP, tag="psu")
            for ks in range(KS):
                nc.tensor.matmul(psu[:], wu_sb[:, ks, m * P:(m + 1) * P], h2[:, ks, :],
                                 start=(ks == 0), stop=(ks == KS - 1))
            sg = spool.tile([P, TQ], FP, tag="sg")
            nc.scalar.activation(out=sg[:], in_=psg[:], func=AF.Silu)
            nc.vector.tensor_tensor(act[:, m, :], sg[:], psu[:], OP.mult)

        # down proj + residual(0.25 * y2), in place into y2
        for m in range(KS):
            psz = psB.tile([P, TQ], FP, tag="psz")
            for ks in range(FFS // P):
                nc.tensor.matmul(psz[:], wd_sb[:, ks, m * P:(m + 1) * P], act[:, ks, :],
                                 start=(ks == 0), stop=(ks == FFS // P - 1))
            nc.vector.tensor_scalar(out=y2[:, m, :], in0=y2[:, m, :], scalar1=0.25,
                                    scalar2=None, op0=OP.mult)
            nc.vector.tensor_tensor(y2[:, m, :], y2[:, m, :], psz[:], OP.add)
        nc.sync.dma_start(out=out_r[:, :, cs], in_=y2[:])


# ---------------- host side ----------------

def _rope_tiles():
    inv_freq = 1.0 / (ROPE_BASE ** (np.arange(0, HD, 2, dtype=np.float32) / HD))
    freqs = np.arange(T, dtype=np.float32)[:, None] * inv_freq[None, :]  # [T, 32]
    cos = np.cos(freqs).astype(np.float32)
    sin = np.sin(freqs).astype(np.float32)
    # tile 4x along partitions for 4 heads: [128, T]
    cosT = np.tile(cos.T, (HG, 1))
    sinT = np.tile(sin.T, (HG, 1))
    return np.ascontiguousarray(cosT), np.ascontiguousarray(sinT)


def _lohi_perm():
    # per-head de-interleave, globally grouped: [h0..h3 lo(32) | h0..h3 hi(32)]
    idx = []
    for h in range(HG):
        idx.extend(range(h * HD, h * HD + HD, 2))      # lo of head h
    for h in range(HG):
        idx.extend(range(h * HD + 1, h * HD + HD, 2))  # hi of head h
    return np.array(idx)  # len 256, indexes into a [HG*HD] block


def kernel(x, mask, norm1_w, Wqkv, bqkv, Wo, bo, norm2_w, Wgate, Wup, Wdown):
    x = np.asarray(x, dtype=np.float32)
    norm1_w = np.asarray(norm1_w, np.float32)
    Wqkv = np.asarray(Wqkv, np.float32)
    bqkv = np.asarray(bqkv, np.float32)
    Wo_ = np.asarray(Wo, np.float32)
    bo_ = np.asarray(bo, np.float32)
    norm2_w = np.asarray(norm2_w, np.float32)
    Wgate = np.asarray(Wgate, np.float32)
    Wup = np.asarray(Wup, np.float32)
    Wdown = np.asarray(Wdown, np.float32)

    if "nc" not in _CACHE:
        _CACHE["nc"] = _build_nc()
    nc = _CACHE["nc"]

    cosT, sinT = _rope_tiles()
    perm = _lohi_perm()

    # fold norm weights into the matmul weights
    Wqkv_f = Wqkv * norm1_w[:, None]
    Wg_f = Wgate * norm2_w[:, None]
    Wu_f = Wup * norm2_w[:, None]

    Wq = Wqkv_f[:, 0:D]
    Wk = Wqkv_f[:, D:2 * D]
    Wv = Wqkv_f[:, 2 * D:3 * D]
    bq = bqkv[0:D]
    bk = bqkv[D:2 * D]
    bv = bqkv[2 * D:3 * D]

    # host rmsnorm1 stats
    rstd1 = 1.0 / np.sqrt(np.mean(x * x, axis=-1) + EPS)  # [B, T]

    in_maps = []
    for c in range(NCORES):
        b = c // TPG
        g = c % TPG
        hs = slice(g * HG * HD, (g + 1) * HG * HD)   # this core's head cols
        fs = slice(g * FFS, (g + 1) * FFS)

        wq_g = Wq[:, hs][:, perm]   # [1024, 256] lo|hi permuted
        wk_g = Wk[:, hs][:, perm]
        bq_g = bq[hs][perm]
        bk_g = bk[hs][perm]
        wqk_g = np.concatenate([wq_g, wk_g], axis=1)           # [1024, 512]
        bqk_g = np.concatenate([bq_g, bk_g], axis=0)           # [512]

        in_maps.append({
            "xT": np.ascontiguousarray(x[b].T),
            "rstd1": np.ascontiguousarray(rstd1[b][None, :]),
            "cosT": cosT,
            "sinT": sinT,
            "wqk": np.ascontiguousarray(wqk_g),
            "bqk": np.ascontiguousarray(bqk_g),
            "wv": np.ascontiguousarray(Wv[:, hs]),
            "bv": np.ascontiguousarray(bv[hs][None, :]),
            "wo": np.ascontiguousarray(Wo_[hs, :]),
            "bo": bo_,
            "wg": np.ascontiguousarray(Wg_f[:, fs]),
            "wu": np.ascontiguousarray(Wu_f[:, fs]),
            "wd": np.ascontiguousarray(Wdown[fs, :]),
        })

    res = run_bass_kernel_spmd(nc, in_maps, core_ids=list(range(NCORES)),
                               **_CACHE.get("run_kwargs", {}))
    _CACHE["last_res"] = res

    out = np.empty((B, T, D), dtype=np.float32)
    for b in range(B):
        acc = res.results[b * TPG]["outT"].astype(np.float32)
        for g in range(1, TPG):
            acc = acc + res.results[b * TPG + g]["outT"]
        out[b] = acc.T
    return out



# revision 2
# speedup vs baseline: 1.0007x; 1.0007x over previous
"""Llama decoder block on 8 trn2 NeuronCores.

Sharding: DP2 x TP4 (core c -> batch c//4, group g=c%4 of 4 heads / 1024 d_ff
columns). One on-device AllReduce (groups [[0-3],[4-7]]) after out_proj; the
post-down-proj reduction is done on the host (partials summed per batch).

Device dataflow (per core, everything feature-major so all linear matmuls
contract on the partition dim with zero on-device transposes):
  h.T   = x.T * rstd1 (rstd1 host-precomputed; norm1_w folded into weights)
  qk.T  = Wqk.T-as-lhsT @ h.T   (cols host-permuted to [q_lo|q_hi|k_lo|k_hi])
  v     = h.T-as-lhsT @ Wv      (token-major, augmented with a ones column)
  RoPE on qk.T via host cos/sin tiles, repack per head via SBUF-SBUF DMA
  scores.T = k_h-as-lhsT @ q_h  ([tk,tq]; exp with scale=1/8, no max needed)
  ctx.T | l = expT-as-lhsT? no: lhsT=v_aug, rhs=expT -> [65, tq] psum accum
  y.T partial = Wo-as-lhsT @ ctx.T -> AllReduce
  y2.T  = AR + x.T + bo;  h2.T = y2.T * rstd2 (norm2_w folded into Wg/Wu)
  act.T = silu(Wg.T@h2.T) * (Wu.T@h2.T);  z.T = Wd.T@act.T
  out_partial.T = z.T + 0.25*y2.T  (host sums 4 partials per batch)
"""

import numpy as np
from contextlib import ExitStack

import concourse.bass as bass
import concourse.tile as tile
from concourse import bacc, mybir
from concourse.bass_utils import run_bass_kernel_spmd

# model dims (hardcoded per problem spec)
D = 1024
H = 16
HD = 64
DFF = 4096
B = 2
T = 2048
EPS = 1e-6
ROPE_BASE = 10000.0

NCORES = 8
TPG = 4              # tensor-parallel group size
HG = H // TPG        # 4 heads per core
QKW = HG * HD * 2    # 512 qk cols per core
VW = HG * HD         # 256 v cols per core
FFS = DFF // TPG     # 1024 ff cols per core
P = 128
KS = D // P          # 8 contraction subtiles for d_model
NTQ = 4
TQ = T // NTQ        # 512-token chunks
NTOK = T // P        # 16 token tiles of 128
FP = mybir.dt.float32

_CACHE = {}


def _build_nc():
    nc = bacc.Bacc("TRN2", target_bir_lowering=False, num_devices=NCORES)

    dt_in = {}
    def din(name, shape):
        dt_in[name] = nc.dram_tensor(name, list(shape), FP, kind="ExternalInput")
        return dt_in[name]

    xT = din("xT", (D, T))            # x[b].T
    rstd1 = din("rstd1", (1, T))
    cosT = din("cosT", (P, T))        # [4 heads x 32 pairs, T]
    sinT = din("sinT", (P, T))
    wqk = din("wqk", (D, QKW))        # cols: [q_lo(128) | q_hi(128) | k_lo | k_hi]
    bqk = din("bqk", (QKW,))
    wv = din("wv", (D, VW))
    bv = din("bv", (1, VW))
    wo = din("wo", (VW, D))           # rows = this core's ctx features
    bo = din("bo", (D,))
    wg = din("wg", (D, FFS))
    wu = din("wu", (D, FFS))
    wd = din("wd", (FFS, D))

    outT = nc.dram_tensor("outT", [D, T], FP, kind="ExternalOutput")

    ar_in = nc.dram_tensor("ar_in", [D, T], FP)
    ar_out = nc.dram_tensor("ar_out", [D, T], FP)

    with tile.TileContext(nc) as tc:
        _body(tc, xT, rstd1, cosT, sinT, wqk, bqk, wv, bv, wo, bo,
              wg, wu, wd, outT, ar_in, ar_out)
    nc.compile()
    return nc


def _body(tc, xT, rstd1, cosT, sinT, wqk, bqk, wv, bv, wo, bo,
          wg, wu, wd, outT, ar_in, ar_out):
    nc = tc.nc
    AF = mybir.ActivationFunctionType
    OP = mybir.AluOpType

    with ExitStack() as ctx:
        singles = ctx.enter_context(tc.tile_pool(name="singles", bufs=1))

        # ---- small persistent loads ----
        bqk_sb = singles.tile([P, QKW // P], FP)   # [p, tile] per-partition scalars
        nc.sync.dma_start(out=bqk_sb[:], in_=bqk.ap().rearrange("(i p) -> p i", p=P))
        bv_sb = singles.tile([P, VW], FP)
        nc.gpsimd.dma_start(out=bv_sb[:], in_=bv.ap().to_broadcast((P, VW)))
        bo_sb = singles.tile([P, KS], FP)
        nc.sync.dma_start(out=bo_sb[:], in_=bo.ap().rearrange("(i p) -> p i", p=P))
        ones_sb = singles.tile([P, 1], FP)
        nc.vector.memset(ones_sb[:], 1.0)
        eps_sb = singles.tile([1, 1], FP)
        nc.vector.memset(eps_sb[:], EPS)

        with nc.named_scope("attn"), ExitStack() as attn_ctx:
            _attn_phase(tc, attn_ctx, singles, xT, rstd1, cosT, sinT,
                        bqk_sb, bv_sb, wqk, wv, wo, ar_in)

        with nc.named_scope("allreduce"):
            nc.gpsimd.collective_compute(
                "AllReduce", OP.add,
                replica_groups=[[0, 1, 2, 3], [4, 5, 6, 7]],
                ins=[ar_in.ap()], outs=[ar_out.ap()],
            )

        with nc.named_scope("mlp"), ExitStack() as mlp_ctx:
            _mlp_phase(tc, mlp_ctx, xT, ar_out, bo_sb, ones_sb, eps_sb,
                       wg, wu, wd, outT)


def _attn_phase(tc, ctx, singles, xT, rstd1, cosT, sinT, bqk_sb, bv_sb,
                wqk, wv, wo, ar_in):
    nc = tc.nc
    AF = mybir.ActivationFunctionType
    OP = mybir.AluOpType

    wpool = ctx.enter_context(tc.tile_pool(name="attn_w", bufs=1))
    rstd1_sb = wpool.tile([P, T], FP)
    nc.gpsimd.dma_start(out=rstd1_sb[:], in_=rstd1.ap().to_broadcast((P, T)))
    cos_sb = wpool.tile([P, T], FP)
    nc.sync.dma_start(out=cos_sb[:], in_=cosT.ap())
    sin_sb = wpool.tile([P, T], FP)
    nc.sync.dma_start(out=sin_sb[:], in_=sinT.ap())
    wqk_sb = wpool.tile([P, KS, QKW], FP)
    nc.sync.dma_start(out=wqk_sb[:], in_=wqk.ap().rearrange("(ks p) m -> p ks m", p=P))
    wv_sb = wpool.tile([P, KS, VW], FP)
    nc.sync.dma_start(out=wv_sb[:], in_=wv.ap().rearrange("(ks p) m -> p ks m", p=P))
    wo_sb = wpool.tile([P, VW // P, D], FP)
    nc.sync.dma_start(out=wo_sb[:], in_=wo.ap().rearrange("(ks p) m -> p ks m", p=P))

    persist = ctx.enter_context(tc.tile_pool(name="attn_persist", bufs=1))
    # v storage token-major, per head slot of 66 cols: [v(64) | 1.0 | pad]
    vt = persist.tile([P, NTOK, HG, 66], FP)
    nc.vector.memset(vt[:, :, :, 64:65], 1.0)
    # rope'd per-head q/k: 2 tiles each holding 2 heads at partition 64*h
    qh = [persist.tile([P, T], FP, name=f"qh{i}") for i in range(2)]
    kh = [persist.tile([P, T], FP, name=f"kh{i}") for i in range(2)]
    # ctx.T accumulated [256, T] as [128, 2, T]
    ctxT = persist.tile([P, 2, T], FP)

    xpool = ctx.enter_context(tc.tile_pool(name="attn_x", bufs=2))
    qkpool = ctx.enter_context(tc.tile_pool(name="attn_qk", bufs=2))
    tpool = ctx.enter_context(tc.tile_pool(name="attn_tmp", bufs=2))
    epool = ctx.enter_context(tc.tile_pool(name="attn_exp", bufs=4))
    opool = ctx.enter_context(tc.tile_pool(name="attn_out", bufs=3))
    small = ctx.enter_context(tc.tile_pool(name="attn_small", bufs=2))

    psA = ctx.enter_context(tc.tile_pool(name="psA", bufs=2, space="PSUM"))
    psB = ctx.enter_context(tc.tile_pool(name="psB", bufs=2, space="PSUM"))
    psC = ctx.enter_context(tc.tile_pool(name="psC", bufs=2, space="PSUM"))

    # ---- qkv + rope + repack, chunked over 512 tokens ----
    for c in range(NTQ):
        cs = slice(c * TQ, (c + 1) * TQ)
        ht = xpool.tile([P, KS, TQ], FP, tag="ht")
        nc.sync.dma_start(
            out=ht[:],
            in_=xT.ap().rearrange("(ks p) t -> p ks t", p=P)[:, :, cs],
        )
        # h.T = x.T * rstd1 (broadcast along partitions), in place
        for ks in range(KS):
            nc.vector.tensor_tensor(
                ht[:, ks, :], ht[:, ks, :], rstd1_sb[:, cs], OP.mult,
            )

        # qk.T chunk: 4 psum tiles of [128, 512]
        qkc = qkpool.tile([P, 4, TQ], FP, tag="qkc")  # [qlo,qhi,klo,khi] chunk
        for m in range(4):
            ps = psB.tile([P, TQ], FP, tag="mm")
            for ks in range(KS):
                nc.tensor.matmul(ps[:], wqk_sb[:, ks, m * P:(m + 1) * P], ht[:, ks, :],
                                 start=(ks == 0), stop=(ks == KS - 1))
            # copy psum -> sbuf with bias add (per-partition scalar)
            nc.vector.tensor_scalar(
                out=qkc[:, m, :], in0=ps[:], scalar1=bqk_sb[:, m:m + 1], scalar2=None,
                op0=OP.add,
            )

        # v chunk: token-major
        for jj in range(TQ // P):
            j = c * (TQ // P) + jj
            psv_full = psB.tile([P, TQ], FP, tag="mm", name="psv")
            psv = psv_full[:, :VW]
            for ks in range(KS):
                nc.tensor.matmul(psv[:], ht[:, ks, jj * P:(jj + 1) * P], wv_sb[:, ks, :],
                                 start=(ks == 0), stop=(ks == KS - 1))
            nc.vector.tensor_tensor(
                vt[:, j, :, 0:64],
                psv.rearrange("p (h d) -> p h d", h=HG),
                bv_sb.rearrange("p (h d) -> p h d", h=HG),
                OP.add,
            )

        # rope on qk chunk: out_lo = lo*cos - hi*sin ; out_hi = lo*sin + hi*cos
        cs_cos = cos_sb[:, cs]
        cs_sin = sin_sb[:, cs]
        for pair in range(2):  # 0 = q, 1 = k
            lo = qkc[:, 2 * pair, :]
            hi = qkc[:, 2 * pair + 1, :]
            t_lo = tpool.tile([P, TQ], FP, tag="t_lo")
            t_hi = tpool.tile([P, TQ], FP, tag="t_hi")
            t3 = tpool.tile([P, TQ], FP, tag="t3")
            nc.vector.tensor_tensor(t_lo[:], lo, cs_cos, OP.mult)
            nc.vector.tensor_tensor(t3[:], hi, cs_sin, OP.mult)
            nc.vector.tensor_tensor(t_lo[:], t_lo[:], t3[:], OP.subtract)
            nc.vector.tensor_tensor(t_hi[:], lo, cs_sin, OP.mult)
            nc.vector.tensor_tensor(t3[:], hi, cs_cos, OP.mult)
            nc.vector.tensor_tensor(t_hi[:], t_hi[:], t3[:], OP.add)
            # repack: head h -> (tile h//2, partitions 64*(h%2) + [0:32 lo | 32:64 hi])
            dst = qh if pair == 0 else kh
            for h in range(HG):
                po = 64 * (h % 2)
                nc.sync.dma_start(out=dst[h // 2][po:po + 32, cs], in_=t_lo[32 * h:32 * h + 32, :])
                nc.sync.dma_start(out=dst[h // 2][po + 32:po + 64, cs], in_=t_hi[32 * h:32 * h + 32, :])

    # ---- attention: scores.T -> exp -> mask -> pv ----
    for c in range(NTQ):
        qs = slice(c * TQ, (c + 1) * TQ)
        for h in range(HG):
            po = 64 * (h % 2)
            qtile = qh[h // 2]
            ktile = kh[h // 2]
            pctx = psC.tile([65, TQ], FP, tag="pctx")
            nblk = 4 * c + 4
            for blk in range(nblk):
                pscore = psA.tile([P, TQ], FP, tag="pscore")
                nc.tensor.matmul(
                    pscore[:],
                    ktile[po:po + 64, blk * P:(blk + 1) * P],
                    qtile[po:po + 64, qs],
                    start=True, stop=True,
                )
                et = epool.tile([P, TQ], FP, tag="et")
                nc.scalar.activation(out=et[:], in_=pscore[:], func=AF.Exp, scale=0.125)
                r = blk - 4 * c
                if r >= 0:
                    # keep iff tq >= tk: j - p - 128r >= 0 else 0
                    nc.gpsimd.affine_select(
                        out=et[:], in_=et[:], compare_op=OP.is_ge, fill=0.0,
                        base=-P * r, channel_multiplier=-1, pattern=[[1, TQ]],
                    )
                nc.tensor.matmul(pctx[:], vt[:, blk, h, 0:65], et[:],
                                 start=(blk == 0), stop=(blk == nblk - 1))
            rl = small.tile([1, TQ], FP, tag="rl")
            nc.vector.reciprocal(rl[:], pctx[64:65, :])
            rlb = small.tile([64, TQ], FP, tag="rlb")
            nc.gpsimd.partition_broadcast(rlb[:], rl[0:1, :])
            nc.vector.tensor_tensor(
                ctxT[po:po + 64, h // 2, qs], pctx[0:64, :], rlb[:], OP.mult,
            )

    # ---- out_proj partial -> ar_in ----
    ar_in_r = ar_in.ap().rearrange("(m p) t -> p m t", p=P)
    for m in range(KS):
        for c in range(NTQ):
            pso = psB.tile([P, TQ], FP, tag="pso")
            for k2 in range(VW // P):
                nc.tensor.matmul(pso[:], wo_sb[:, k2, m * P:(m + 1) * P],
                                 ctxT[:, k2, c * TQ:(c + 1) * TQ],
                                 start=(k2 == 0), stop=(k2 == VW // P - 1))
            yo = opool.tile([P, TQ], FP, tag="yo")
            nc.vector.tensor_copy(out=yo[:], in_=pso[:])
            nc.sync.dma_start(out=ar_in_r[:, m, c * TQ:(c + 1) * TQ], in_=yo[:])


def _mlp_phase(tc, ctx, xT, ar_out, bo_sb, ones_sb, eps_sb, wg, wu, wd, outT):
    nc = tc.nc
    AF = mybir.ActivationFunctionType
    OP = mybir.AluOpType

    wpool = ctx.enter_context(tc.tile_pool(name="mlp_w", bufs=1))
    wg_sb = wpool.tile([P, KS, FFS], FP)
    nc.sync.dma_start(out=wg_sb[:], in_=wg.ap().rearrange("(ks p) m -> p ks m", p=P))
    wu_sb = wpool.tile([P, KS, FFS], FP)
    nc.sync.dma_start(out=wu_sb[:], in_=wu.ap().rearrange("(ks p) m -> p ks m", p=P))
    wd_sb = wpool.tile([P, FFS // P, D], FP)
    nc.sync.dma_start(out=wd_sb[:], in_=wd.ap().rearrange("(ks p) m -> p ks m", p=P))

    cpool = ctx.enter_context(tc.tile_pool(name="mlp_c", bufs=1))
    scpool = ctx.enter_context(tc.tile_pool(name="mlp_scratch", bufs=2))
    spool = ctx.enter_context(tc.tile_pool(name="mlp_s", bufs=2))
    psA = ctx.enter_context(tc.tile_pool(name="psMA", bufs=2, space="PSUM"))
    psB = ctx.enter_context(tc.tile_pool(name="psMB", bufs=2, space="PSUM"))
    psS = ctx.enter_context(tc.tile_pool(name="psMS", bufs=1, space="PSUM"))

    xT_r = xT.ap().rearrange("(ks p) t -> p ks t", p=P)
    ar_r = ar_out.ap().rearrange("(ks p) t -> p ks t", p=P)
    out_r = outT.ap().rearrange("(m p) t -> p m t", p=P)

    for c in range(NTQ):
        cs = slice(c * TQ, (c + 1) * TQ)
        y2 = cpool.tile([P, KS, TQ], FP, tag="y2")
        xc = scpool.tile([P, KS, TQ], FP, tag="scratch", name="xc")
        nc.sync.dma_start(out=y2[:], in_=ar_r[:, :, cs])
        nc.sync.dma_start(out=xc[:], in_=xT_r[:, :, cs])
        nc.vector.tensor_tensor(y2[:], y2[:], xc[:], OP.add)
        for ks in range(KS):
            nc.vector.tensor_scalar(
                out=y2[:, ks, :], in0=y2[:, ks, :],
                scalar1=bo_sb[:, ks:ks + 1], scalar2=None, op0=OP.add)

        # rmsnorm2: sumsq over features (partitions) via ones-matmul
        sq = scpool.tile([P, KS, TQ], FP, tag="scratch", name="sq")
        nc.vector.tensor_tensor(sq[:], y2[:], y2[:], OP.mult)
        pss = psS.tile([1, TQ], FP, tag="pss")
        for ks in range(KS):
            nc.tensor.matmul(pss[:], ones_sb[:], sq[:, ks, :],
                             start=(ks == 0), stop=(ks == KS - 1))
        rstd2 = spool.tile([1, TQ], FP, tag="rstd2")
        nc.scalar.activation(out=rstd2[:], in_=pss[:], func=AF.Sqrt,
                             bias=eps_sb[0:1, 0:1], scale=1.0 / D)
        nc.vector.reciprocal(rstd2[:], rstd2[:])
        rstd2_b = spool.tile([P, TQ], FP, tag="rstd2_b")
        nc.gpsimd.partition_broadcast(rstd2_b[:], rstd2[0:1, :])

        h2 = cpool.tile([P, KS, TQ], FP, tag="h2")
        for ks in range(KS):
            nc.vector.tensor_tensor(
                h2[:, ks, :], y2[:, ks, :], rstd2_b[:], OP.mult)

        # gate/up -> act (feature-major over ff shard)
        act = cpool.tile([P, FFS // P, TQ], FP, tag="act")
        for m in range(FFS // P):
            psg = psA.tile([P, TQ], FP, tag="psg")
            for ks in range(KS):
                nc.tensor.matmul(psg[:], wg_sb[:, ks, m * P:(m + 1) * P], h2[:, ks, :],
                                 start=(ks == 0), stop=(ks == KS - 1))
            psu = psB.tile([P, TQ], FP, tag="psu")
            for ks in range(KS):
                nc.tensor.matmul(psu[:], wu_sb[:, ks, m * P:(m + 1) * P], h2[:, ks, :],
                                 start=(ks == 0), stop=(ks == KS - 1))
            sg = spool.tile([P, TQ], FP, tag="sg")
            nc.scalar.activation(out=sg[:], in_=psg[:], func=AF.Silu)
            nc.vector.tensor_tensor(act[:, m, :], sg[:], psu[:], OP.mult)

        # down proj + residual(0.25 * y2), in place into y2
        for m in range(KS):
            psz = psB.tile([P, TQ], FP, tag="psz")
            for ks in range(FFS // P):
                nc.tensor.matmul(psz[:], wd_sb[:, ks, m * P:(m + 1) * P], act[:, ks, :],
                                 start=(ks == 0), stop=(ks == FFS // P - 1))
            nc.vector.tensor_scalar(out=y2[:, m, :], in0=y2[:, m, :], scalar1=0.25,
                                    scalar2=None, op0=OP.mult)
            nc.vector.tensor_tensor(y2[:, m, :], y2[:, m, :], psz[:], OP.add)
        nc.sync.dma_start(out=out_r[:, :, cs], in_=y2[:])


# ---------------- host side ----------------

def _rope_tiles():
    inv_freq = 1.0 / (ROPE_BASE ** (np.arange(0, HD, 2, dtype=np.float32) / HD))
    freqs = np.arange(T, dtype=np.float32)[:, None] * inv_freq[None, :]  # [T, 32]
    cos = np.cos(freqs).astype(np.float32)
    sin = np.sin(freqs).astype(np.float32)
    # tile 4x along partitions for 4 heads: [128, T]
    cosT = np.tile(cos.T, (HG, 1))
    sinT = np.tile(sin.T, (HG, 1))
    return np.ascontiguousarray(cosT), np.ascontiguousarray(sinT)


def _lohi_perm():
    # per-head de-interleave, globally grouped: [h0..h3 lo(32) | h0..h3 hi(32)]
    idx = []
    for h in range(HG):
        idx.extend(range(h * HD, h * HD + HD, 2))      # lo of head h
    for h in range(HG):
        idx.extend(range(h * HD + 1, h * HD + HD, 2))  # hi of head h
    return np.array(idx)  # len 256, indexes into a [HG*HD] block


def kernel(x, mask, norm1_w, Wqkv, bqkv, Wo, bo, norm2_w, Wgate, Wup, Wdown):
    x = np.asarray(x, dtype=np.float32)
    norm1_w = np.asarray(norm1_w, np.float32)
    Wqkv = np.asarray(Wqkv, np.float32)
    bqkv = np.asarray(bqkv, np.float32)
    Wo_ = np.asarray(Wo, np.float32)
    bo_ = np.asarray(bo, np.float32)
    norm2_w = np.asarray(norm2_w, np.float32)
    Wgate = np.asarray(Wgate, np.float32)
    Wup = np.asarray(Wup, np.float32)
    Wdown = np.asarray(Wdown, np.float32)

    if "nc" not in _CACHE:
        _CACHE["nc"] = _build_nc()
    nc = _CACHE["nc"]

    cosT, sinT = _rope_tiles()
    perm = _lohi_perm()

    # fold norm weights into the matmul weights
    Wqkv_f = Wqkv * norm1_w[:, None]
    Wg_f = Wgate * norm2_w[:, None]
    Wu_f = Wup * norm2_w[:, None]

    Wq = Wqkv_f[:, 0:D]
    Wk = Wqkv_f[:, D:2 * D]
    Wv = Wqkv_f[:, 2 * D:3 * D]
    bq = bqkv[0:D]
    bk = bqkv[D:2 * D]
    bv = bqkv[2 * D:3 * D]

    # host rmsnorm1 stats
    rstd1 = 1.0 / np.sqrt(np.mean(x * x, axis=-1) + EPS)  # [B, T]

    in_maps = []
    for c in range(NCORES):
        b = c // TPG
        g = c % TPG
        hs = slice(g * HG * HD, (g + 1) * HG * HD)   # this core's head cols
        fs = slice(g * FFS, (g + 1) * FFS)

        wq_g = Wq[:, hs][:, perm]   # [1024, 256] lo|hi permuted
        wk_g = Wk[:, hs][:, perm]
        bq_g = bq[hs][perm]
        bk_g = bk[hs][perm]
        wqk_g = np.concatenate([wq_g, wk_g], axis=1)           # [1024, 512]
        bqk_g = np.concatenate([bq_g, bk_g], axis=0)           # [512]

        in_maps.append({
            "xT": np.ascontiguousarray(x[b].T),
            "rstd1": np.ascontiguousarray(rstd1[b][None, :]),
            "cosT": cosT,
            "sinT": sinT,
            "wqk": np.ascontiguousarray(wqk_g),
            "bqk": np.ascontiguousarray(bqk_g),
            "wv": np.ascontiguousarray(Wv[:, hs]),
            "bv": np.ascontiguousarray(bv[hs][None, :]),
            "wo": np.ascontiguousarray(Wo_[hs, :]),
            "bo": bo_,
            "wg": np.ascontiguousarray(Wg_f[:, fs]),
            "wu": np.ascontiguousarray(Wu_f[:, fs]),
            "wd": np.ascontiguousarray(Wdown[fs, :]),
        })

    res = run_bass_kernel_spmd(nc, in_maps, core_ids=list(range(NCORES)),
                               **_CACHE.get("run_kwargs", {}))
    _CACHE["last_res"] = res

    out = np.empty((B, T, D), dtype=np.float32)
    for b in range(B):
        acc = res.results[b * TPG]["outT"].astype(np.float32)
        for g in range(1, TPG):
            acc = acc + res.results[b * TPG + g]["outT"]
        out[b] = acc.T
    return out



# revision 41
# speedup vs baseline: 1.8564x; 1.8551x over previous
"""Llama decoder block on 8 trn2 NeuronCores.

Sharding: DP2 x TP4 (core c -> batch c//4, group g=c%4 of 4 heads / 1024 d_ff
columns). One on-device AllReduce (groups [[0-3],[4-7]]) after out_proj; the
post-down-proj reduction is done on the host (partials summed per batch).

Device dataflow (per core, everything feature-major so all linear matmuls
contract on the partition dim with zero on-device transposes):
  h.T   = x.T * rstd1 (rstd1 host-precomputed; norm1_w folded into weights)
  qk.T  = Wqk.T-as-lhsT @ h.T   (cols host-permuted to [q_lo|q_hi|k_lo|k_hi])
  v     = h.T-as-lhsT @ Wv      (token-major, augmented with a ones column)
  RoPE on qk.T via host cos/sin tiles, repack per head via SBUF-SBUF DMA
  scores.T = k_h-as-lhsT @ q_h  ([tk,tq]; exp with scale=1/8, no max needed)
  ctx.T | l = expT-as-lhsT? no: lhsT=v_aug, rhs=expT -> [65, tq] psum accum
  y.T partial = Wo-as-lhsT @ ctx.T -> AllReduce
  y2.T  = AR + x.T + bo;  h2.T = y2.T * rstd2 (norm2_w folded into Wg/Wu)
  act.T = silu(Wg.T@h2.T) * (Wu.T@h2.T);  z.T = Wd.T@act.T
  out_partial.T = z.T + 0.25*y2.T  (host sums 4 partials per batch)
"""

import numpy as np
from contextlib import ExitStack

import concourse.bass as bass
import concourse.tile as tile
from concourse import bacc, mybir
from concourse.bass_utils import run_bass_kernel_spmd

# model dims (hardcoded per problem spec)
D = 1024
H = 16
HD = 64
DFF = 4096
B = 2
T = 2048
EPS = 1e-6
ROPE_BASE = 10000.0

NCORES = 8
TPG = 4              # tensor-parallel group size
HG = H // TPG        # 4 heads per core
QKW = HG * HD * 2    # 512 qk cols per core
VW = HG * HD         # 256 v cols per core
FFS = DFF // TPG     # 1024 ff cols per core
P = 128
KS = D // P          # 8 contraction subtiles for d_model
NTQ = 4
TQ = T // NTQ        # 512-token chunks
NTOK = T // P        # 16 token tiles of 128
FP = mybir.dt.float32
R = mybir.dt.float32r   # fp32 with ~14-bit mantissa: 1 PE cycle/row vs 4
BF = mybir.dt.bfloat16

_CACHE = {}


def _mmr(nc, out, lhsT, rhs, **kw):
    """Matmul with both operands as float32r (operands' tiles are R-typed)."""
    nc.tensor.matmul(out, lhsT, rhs, **kw)


def _build_nc():
    nc = bacc.Bacc("TRN2", target_bir_lowering=False, num_devices=NCORES)

    dt_in = {}
    def din(name, shape):
        dt_in[name] = nc.dram_tensor(name, list(shape), FP, kind="ExternalInput")
        return dt_in[name]

    xT = din("xT", (D, T))            # x[b].T
    rstd1 = din("rstd1", (1, T))
    cosT = din("cosT", (P, T))        # [4 heads x 32 pairs, T]
    sinT = din("sinT", (P, T))
    wqk = din("wqk", (D, QKW))        # cols: [q_lo(128) | q_hi(128) | k_lo | k_hi]
    bqk = din("bqk", (QKW,))
    wv = din("wv", (D, VW))
    bv = din("bv", (1, VW))
    wo = din("wo", (VW, D))           # rows = this core's ctx features
    bo = din("bo", (D,))
    wg = din("wg", (D, FFS))
    wu = din("wu", (D, FFS))
    wd = din("wd", (FFS, D))
    onesd = din("onesd", (P, NTOK * HG * 2))  # [1,0] pairs (fp32r memset unsupported)
    maskt = nc.dram_tensor("maskt", [P, 4 * TQ], BF, kind="ExternalInput")
    dt_in["maskt"] = maskt                # causal -1e9 mask tiles for r=0..3

    outT = nc.dram_tensor("outT", [D, T], FP, kind="ExternalOutput")

    ar_in = nc.dram_tensor("ar_in", [D, T], FP)
    ar_out = nc.dram_tensor("ar_out", [D, T], FP)

    with tile.TileContext(nc) as tc:
        _body(tc, xT, rstd1, cosT, sinT, wqk, bqk, wv, bv, wo, bo,
              wg, wu, wd, onesd, maskt, outT, ar_in, ar_out)
    nc.compile()
    return nc


def _body(tc, xT, rstd1, cosT, sinT, wqk, bqk, wv, bv, wo, bo,
          wg, wu, wd, onesd, maskt, outT, ar_in, ar_out):
    nc = tc.nc
    AF = mybir.ActivationFunctionType
    OP = mybir.AluOpType

    with ExitStack() as ctx:
        singles = ctx.enter_context(tc.tile_pool(name="singles", bufs=1))

        # ---- small persistent loads ----
        bqk_sb = singles.tile([P, QKW // P], FP)   # [p, tile] per-partition scalars
        nc.sync.dma_start(out=bqk_sb[:], in_=bqk.ap().rearrange("(i p) -> p i", p=P))
        bv_sb = singles.tile([P, VW], FP)
        nc.gpsimd.dma_start(out=bv_sb[:], in_=bv.ap().to_broadcast((P, VW)))
        bo_sb = singles.tile([P, KS], FP)
        nc.sync.dma_start(out=bo_sb[:], in_=bo.ap().rearrange("(i p) -> p i", p=P))
        # [1, 0] per partition: even-width fp32r stationary for the sumsq matmul
        ones_sb = singles.tile([P, 2], R)
        nc.sync.dma_start(out=ones_sb[:], in_=onesd.ap()[:, 0:2].bitcast(R))
        eps_sb = singles.tile([1, 1], FP)
        nc.vector.memset(eps_sb[:], EPS)

        with nc.named_scope("attn"), ExitStack() as attn_ctx:
            _attn_phase(tc, attn_ctx, singles, xT, rstd1, cosT, sinT,
                        bqk_sb, bv_sb, wqk, wv, wo, onesd, maskt, ar_in)

        with nc.named_scope("allreduce"):
            nc.gpsimd.collective_compute(
                "AllReduce", OP.add,
                replica_groups=[[0, 1, 2, 3], [4, 5, 6, 7]],
                ins=[ar_in.ap()], outs=[ar_out.ap()],
            )

        with nc.named_scope("mlp"), ExitStack() as mlp_ctx:
            _mlp_phase(tc, mlp_ctx, xT, ar_out, bo_sb, ones_sb, eps_sb,
                       wg, wu, wd, outT)


def _attn_phase(tc, ctx, singles, xT, rstd1, cosT, sinT, bqk_sb, bv_sb,
                wqk, wv, wo, onesd, maskt, ar_in):
    nc = tc.nc
    AF = mybir.ActivationFunctionType
    OP = mybir.AluOpType

    wpool = ctx.enter_context(tc.tile_pool(name="attn_w", bufs=1))
    rstd1_sb = wpool.tile([P, T], FP)
    nc.gpsimd.dma_start(out=rstd1_sb[:], in_=rstd1.ap().to_broadcast((P, T)))
    cos_sb = wpool.tile([P, T], FP)
    nc.sync.dma_start(out=cos_sb[:], in_=cosT.ap())
    sin_sb = wpool.tile([P, T], FP)
    nc.sync.dma_start(out=sin_sb[:], in_=sinT.ap())
    wqk_sb = wpool.tile([P, KS, QKW], R)
    nc.sync.dma_start(out=wqk_sb[:], in_=wqk.ap().rearrange("(ks p) m -> p ks m", p=P).bitcast(R))
    wv_sb = wpool.tile([P, KS, VW], R)
    nc.sync.dma_start(out=wv_sb[:], in_=wv.ap().rearrange("(ks p) m -> p ks m", p=P).bitcast(R))
    wo_sb = wpool.tile([P, VW // P, D], R)
    nc.sync.dma_start(out=wo_sb[:], in_=wo.ap().rearrange("(ks p) m -> p ks m", p=P).bitcast(R))

    persist = ctx.enter_context(tc.tile_pool(name="attn_persist", bufs=1))
    # v storage token-major, per head slot of 66 cols: [v(64) | 1.0 | pad]
    vt = persist.tile([P, NTOK, HG, 66], R)
    # cols 64:66 = [1.0, 0.0]: col 64 gives the softmax denominator row, col 65
    # pads the pv stationary to an even width (fp32r ISA requirement)
    nc.sync.dma_start(
        out=vt[:, :, :, 64:66],
        in_=onesd.ap().rearrange("p (n h o) -> p n h o", n=NTOK, o=2).bitcast(R),
    )
    mask_sb = wpool.tile([P, 4, TQ], BF)
    nc.sync.dma_start(out=mask_sb[:], in_=maskt.ap().rearrange("p (r t) -> p r t", r=4))
    # rope'd per-head q/k: 2 tiles each holding 2 heads at partition 64*h
    qh = [persist.tile([P, T], R, name=f"qh{i}") for i in range(2)]
    kh = [persist.tile([P, T], R, name=f"kh{i}") for i in range(2)]
    # ctx.T accumulated [256, T] as [128, 2, T]
    ctxT = persist.tile([P, 2, T], R)

    xpool = ctx.enter_context(tc.tile_pool(name="attn_x", bufs=2))
    qkpool = ctx.enter_context(tc.tile_pool(name="attn_qk", bufs=2))
    tpool = ctx.enter_context(tc.tile_pool(name="attn_tmp", bufs=2))
    epool = ctx.enter_context(tc.tile_pool(name="attn_exp", bufs=3))
    opool = ctx.enter_context(tc.tile_pool(name="attn_out", bufs=3))
    small = ctx.enter_context(tc.tile_pool(name="attn_small", bufs=2))

    psA = ctx.enter_context(tc.tile_pool(name="psA", bufs=2, space="PSUM"))
    psB = ctx.enter_context(tc.tile_pool(name="psB", bufs=2, space="PSUM"))
    psC = ctx.enter_context(tc.tile_pool(name="psC", bufs=2, space="PSUM"))

    # ---- qkv + rope + repack, chunked over 512 tokens ----
    for c in range(NTQ):
        cs = slice(c * TQ, (c + 1) * TQ)
        ht = xpool.tile([P, KS, TQ], R, tag="ht")
        nc.sync.dma_start(
            out=ht[:],
            in_=xT.ap().rearrange("(ks p) t -> p ks t", p=P)[:, :, cs].bitcast(R),
        )
        # h.T = x.T * rstd1 (broadcast along partitions), in place
        for ks in range(KS):
            nc.vector.tensor_tensor(
                ht[:, ks, :], ht[:, ks, :], rstd1_sb[:, cs], OP.mult,
            )

        # qk.T chunk: 4 psum tiles of [128, 512]
        qkc = qkpool.tile([P, 4, TQ], FP, tag="qkc")  # [qlo,qhi,klo,khi] chunk
        for m in range(4):
            ps = psB.tile([P, TQ], FP, tag="mm")
            for ks in range(KS):
                _mmr(nc, ps[:], wqk_sb[:, ks, m * P:(m + 1) * P], ht[:, ks, :],
                                 start=(ks == 0), stop=(ks == KS - 1))
            # copy psum -> sbuf with bias add (per-partition scalar)
            nc.vector.tensor_scalar(
                out=qkc[:, m, :], in0=ps[:], scalar1=bqk_sb[:, m:m + 1], scalar2=None,
                op0=OP.add,
            )

        # v chunk: token-major
        for jj in range(TQ // P):
            j = c * (TQ // P) + jj
            psv_full = psB.tile([P, TQ], FP, tag="mm", name="psv")
            psv = psv_full[:, :VW]
            for ks in range(KS):
                _mmr(nc, psv[:], ht[:, ks, jj * P:(jj + 1) * P], wv_sb[:, ks, :],
                                 start=(ks == 0), stop=(ks == KS - 1))
            nc.vector.tensor_tensor(
                vt[:, j, :, 0:64],
                psv.rearrange("p (h d) -> p h d", h=HG),
                bv_sb.rearrange("p (h d) -> p h d", h=HG),
                OP.add,
            )

        # rope on qk chunk: out_lo = lo*cos - hi*sin ; out_hi = lo*sin + hi*cos
        cs_cos = cos_sb[:, cs]
        cs_sin = sin_sb[:, cs]
        for pair in range(2):  # 0 = q, 1 = k
            lo = qkc[:, 2 * pair, :]
            hi = qkc[:, 2 * pair + 1, :]
            t_lo = tpool.tile([P, TQ], R, tag="t_lo")
            t_hi = tpool.tile([P, TQ], R, tag="t_hi")
            t3 = tpool.tile([P, TQ], FP, tag="t3")
            nc.vector.tensor_tensor(t_lo[:], lo, cs_cos, OP.mult)
            nc.vector.tensor_tensor(t3[:], hi, cs_sin, OP.mult)
            nc.vector.tensor_tensor(t_lo[:], t_lo[:], t3[:], OP.subtract)
            nc.vector.tensor_tensor(t_hi[:], lo, cs_sin, OP.mult)
            nc.vector.tensor_tensor(t3[:], hi, cs_cos, OP.mult)
            nc.vector.tensor_tensor(t_hi[:], t_hi[:], t3[:], OP.add)
            # repack: head h -> (tile h//2, partitions 64*(h%2) + [0:32 lo | 32:64 hi])
            dst = qh if pair == 0 else kh
            for h in range(HG):
                po = 64 * (h % 2)
                nc.sync.dma_start(out=dst[h // 2][po:po + 32, cs], in_=t_lo[32 * h:32 * h + 32, :])
                nc.sync.dma_start(out=dst[h // 2][po + 32:po + 64, cs], in_=t_hi[32 * h:32 * h + 32, :])

    # ---- attention: scores.T -> exp -> mask -> pv ----
    for c in range(NTQ):
        qs = slice(c * TQ, (c + 1) * TQ)
        for h in range(HG):
            po = 64 * (h % 2)
            qtile = qh[h // 2]
            ktile = kh[h // 2]
            pctx = psC.tile([66, TQ], FP, tag="pctx")
            nblk = 4 * c + 4
            for blk in range(nblk):
                pscore = psA.tile([P, TQ], FP, tag="pscore")
                _mmr(nc,
                    pscore[:],
                    ktile[po:po + 64, blk * P:(blk + 1) * P],
                    qtile[po:po + 64, qs],
                    start=True, stop=True,
                )
                r = blk - 4 * c
                if r >= 0:
                    # causal: add -1e9 where tq < tk (j < p + 128r), pre-exp
                    nc.vector.tensor_tensor(
                        pscore[:], pscore[:], mask_sb[:, r, :], OP.add,
                    )
                et = epool.tile([P, TQ], R, tag="et")
                nc.scalar.activation(out=et[:], in_=pscore[:], func=AF.Exp, scale=0.125)
                _mmr(nc, pctx[:], vt[:, blk, h, 0:66], et[:],
                                 start=(blk == 0), stop=(blk == nblk - 1))
            rl = small.tile([1, TQ], FP, tag="rl")
            nc.vector.reciprocal(rl[:], pctx[64:65, :])
            rlb = small.tile([64, TQ], FP, tag="rlb")
            nc.gpsimd.partition_broadcast(rlb[:], rl[0:1, :])
            nc.vector.tensor_tensor(
                ctxT[po:po + 64, h // 2, qs], pctx[0:64, :], rlb[:], OP.mult,
            )

    # ---- out_proj partial -> ar_in ----
    ar_in_r = ar_in.ap().rearrange("(m p) t -> p m t", p=P)
    for m in range(KS):
        for c in range(NTQ):
            pso = psB.tile([P, TQ], FP, tag="pso")
            for k2 in range(VW // P):
                _mmr(nc, pso[:], wo_sb[:, k2, m * P:(m + 1) * P],
                                 ctxT[:, k2, c * TQ:(c + 1) * TQ],
                                 start=(k2 == 0), stop=(k2 == VW // P - 1))
            yo = opool.tile([P, TQ], FP, tag="yo")
            nc.vector.tensor_copy(out=yo[:], in_=pso[:])
            nc.sync.dma_start(out=ar_in_r[:, m, c * TQ:(c + 1) * TQ], in_=yo[:])


def _mlp_phase(tc, ctx, xT, ar_out, bo_sb, ones_sb, eps_sb, wg, wu, wd, outT):
    nc = tc.nc
    AF = mybir.ActivationFunctionType
    OP = mybir.AluOpType

    wpool = ctx.enter_context(tc.tile_pool(name="mlp_w", bufs=1))
    wg_sb = wpool.tile([P, KS, FFS], R)
    nc.sync.dma_start(out=wg_sb[:], in_=wg.ap().rearrange("(ks p) m -> p ks m", p=P).bitcast(R))
    wu_sb = wpool.tile([P, KS, FFS], R)
    nc.sync.dma_start(out=wu_sb[:], in_=wu.ap().rearrange("(ks p) m -> p ks m", p=P).bitcast(R))
    wd_sb = wpool.tile([P, FFS // P, D], R)
    nc.sync.dma_start(out=wd_sb[:], in_=wd.ap().rearrange("(ks p) m -> p ks m", p=P).bitcast(R))

    cpool = ctx.enter_context(tc.tile_pool(name="mlp_c", bufs=1))
    scpool = ctx.enter_context(tc.tile_pool(name="mlp_scratch", bufs=2))
    spool = ctx.enter_context(tc.tile_pool(name="mlp_s", bufs=2))
    psA = ctx.enter_context(tc.tile_pool(name="psMA", bufs=2, space="PSUM"))
    psB = ctx.enter_context(tc.tile_pool(name="psMB", bufs=2, space="PSUM"))
    psS = ctx.enter_context(tc.tile_pool(name="psMS", bufs=1, space="PSUM"))

    xT_r = xT.ap().rearrange("(ks p) t -> p ks t", p=P)
    ar_r = ar_out.ap().rearrange("(ks p) t -> p ks t", p=P)
    out_r = outT.ap().rearrange("(m p) t -> p m t", p=P)

    for c in range(NTQ):
        cs = slice(c * TQ, (c + 1) * TQ)
        y2 = cpool.tile([P, KS, TQ], FP, tag="y2")
        xc = scpool.tile([P, KS, TQ], FP, tag="scratch", name="xc")
        nc.sync.dma_start(out=y2[:], in_=ar_r[:, :, cs])
        nc.sync.dma_start(out=xc[:], in_=xT_r[:, :, cs])
        nc.vector.tensor_tensor(y2[:], y2[:], xc[:], OP.add)
        for ks in range(KS):
            nc.vector.tensor_scalar(
                out=y2[:, ks, :], in0=y2[:, ks, :],
                scalar1=bo_sb[:, ks:ks + 1], scalar2=None, op0=OP.add)

        # rmsnorm2: sumsq over features (partitions) via ones-matmul
        sq = scpool.tile([P, KS, TQ], R, tag="scratch", name="sq")
        nc.vector.tensor_tensor(sq[:], y2[:], y2[:], OP.mult)
        pss = psS.tile([2, TQ], FP, tag="pss")
        for ks in range(KS):
            _mmr(nc, pss[:], ones_sb[:], sq[:, ks, :],
                             start=(ks == 0), stop=(ks == KS - 1))
        rstd2 = spool.tile([1, TQ], FP, tag="rstd2")
        nc.scalar.activation(out=rstd2[:], in_=pss[0:1, :], func=AF.Sqrt,
                             bias=eps_sb[0:1, 0:1], scale=1.0 / D)
        nc.vector.reciprocal(rstd2[:], rstd2[:])
        rstd2_b = spool.tile([P, TQ], FP, tag="rstd2_b")
        nc.gpsimd.partition_broadcast(rstd2_b[:], rstd2[0:1, :])

        h2 = cpool.tile([P, KS, TQ], R, tag="h2")
        for ks in range(KS):
            nc.vector.tensor_tensor(
                h2[:, ks, :], y2[:, ks, :], rstd2_b[:], OP.mult)

        # gate/up -> act (feature-major over ff shard)
        act = cpool.tile([P, FFS // P, TQ], R, tag="act")
        for m in range(FFS // P):
            psg = psA.tile([P, TQ], FP, tag="psg")
            for ks in range(KS):
                _mmr(nc, psg[:], wg_sb[:, ks, m * P:(m + 1) * P], h2[:, ks, :],
                                 start=(ks == 0), stop=(ks == KS - 1))
            psu = psB.tile([P, TQ], FP, tag="psu")
            for ks in range(KS):
                _mmr(nc, psu[:], wu_sb[:, ks, m * P:(m + 1) * P], h2[:, ks, :],
                                 start=(ks == 0), stop=(ks == KS - 1))
            sg = spool.tile([P, TQ], FP, tag="sg")
            nc.scalar.activation(out=sg[:], in_=psg[:], func=AF.Silu)
            nc.vector.tensor_tensor(act[:, m, :], sg[:], psu[:], OP.mult)

        # down proj + residual(0.25 * y2), in place into y2
        for m in range(KS):
            psz = psB.tile([P, TQ], FP, tag="psz")
            for ks in range(FFS // P):
                _mmr(nc, psz[:], wd_sb[:, ks, m * P:(m + 1) * P], act[:, ks, :],
                                 start=(ks == 0), stop=(ks == FFS // P - 1))
            nc.vector.tensor_scalar(out=y2[:, m, :], in0=y2[:, m, :], scalar1=0.25,
                                    scalar2=None, op0=OP.mult)
            nc.vector.tensor_tensor(y2[:, m, :], y2[:, m, :], psz[:], OP.add)
        nc.sync.dma_start(out=out_r[:, :, cs], in_=y2[:])


# ---------------- host side ----------------

def _rope_tiles():
    inv_freq = 1.0 / (ROPE_BASE ** (np.arange(0, HD, 2, dtype=np.float32) / HD))
    freqs = np.arange(T, dtype=np.float32)[:, None] * inv_freq[None, :]  # [T, 32]
    cos = np.cos(freqs).astype(np.float32)
    sin = np.sin(freqs).astype(np.float32)
    # tile 4x along partitions for 4 heads: [128, T]
    cosT = np.tile(cos.T, (HG, 1))
    sinT = np.tile(sin.T, (HG, 1))
    return np.ascontiguousarray(cosT), np.ascontiguousarray(sinT)


def _lohi_perm():
    # per-head de-interleave, globally grouped: [h0..h3 lo(32) | h0..h3 hi(32)]
    idx = []
    for h in range(HG):
        idx.extend(range(h * HD, h * HD + HD, 2))      # lo of head h
    for h in range(HG):
        idx.extend(range(h * HD + 1, h * HD + HD, 2))  # hi of head h
    return np.array(idx)  # len 256, indexes into a [HG*HD] block


def kernel(x, mask, norm1_w, Wqkv, bqkv, Wo, bo, norm2_w, Wgate, Wup, Wdown):
    x = np.asarray(x, dtype=np.float32)
    norm1_w = np.asarray(norm1_w, np.float32)
    Wqkv = np.asarray(Wqkv, np.float32)
    bqkv = np.asarray(bqkv, np.float32)
    Wo_ = np.asarray(Wo, np.float32)
    bo_ = np.asarray(bo, np.float32)
    norm2_w = np.asarray(norm2_w, np.float32)
    Wgate = np.asarray(Wgate, np.float32)
    Wup = np.asarray(Wup, np.float32)
    Wdown = np.asarray(Wdown, np.float32)

    if "nc" not in _CACHE:
        _CACHE["nc"] = _build_nc()
    nc = _CACHE["nc"]

    cosT, sinT = _rope_tiles()
    perm = _lohi_perm()

    # fold norm weights into the matmul weights
    Wqkv_f = Wqkv * norm1_w[:, None]
    Wg_f = Wgate * norm2_w[:, None]
    Wu_f = Wup * norm2_w[:, None]

    Wq = Wqkv_f[:, 0:D]
    Wk = Wqkv_f[:, D:2 * D]
    Wv = Wqkv_f[:, 2 * D:3 * D]
    bq = bqkv[0:D]
    bk = bqkv[D:2 * D]
    bv = bqkv[2 * D:3 * D]

    # host rmsnorm1 stats
    rstd1 = 1.0 / np.sqrt(np.mean(x * x, axis=-1) + EPS)  # [B, T]

    onesd_host = np.tile(np.array([1.0, 0.0], np.float32), (P, NTOK * HG))
    p_idx = np.arange(P)[:, None]
    j_idx = np.arange(TQ)[None, :]
    import ml_dtypes
    maskt_host = np.concatenate(
        [np.where(j_idx >= p_idx + P * r, 0.0, -1e9).astype(np.float32)
         for r in range(4)], axis=1)
    maskt_host = np.ascontiguousarray(maskt_host).astype(ml_dtypes.bfloat16)

    in_maps = []
    for c in range(NCORES):
        b = c // TPG
        g = c % TPG
        hs = slice(g * HG * HD, (g + 1) * HG * HD)   # this core's head cols
        fs = slice(g * FFS, (g + 1) * FFS)

        wq_g = Wq[:, hs][:, perm]   # [1024, 256] lo|hi permuted
        wk_g = Wk[:, hs][:, perm]
        bq_g = bq[hs][perm]
        bk_g = bk[hs][perm]
        wqk_g = np.concatenate([wq_g, wk_g], axis=1)           # [1024, 512]
        bqk_g = np.concatenate([bq_g, bk_g], axis=0)           # [512]

        in_maps.append({
            "xT": np.ascontiguousarray(x[b].T),
            "rstd1": np.ascontiguousarray(rstd1[b][None, :]),
            "cosT": cosT,
            "sinT": sinT,
            "wqk": np.ascontiguousarray(wqk_g),
            "bqk": np.ascontiguousarray(bqk_g),
            "wv": np.ascontiguousarray(Wv[:, hs]),
            "bv": np.ascontiguousarray(bv[hs][None, :]),
            "wo": np.ascontiguousarray(Wo_[hs, :]),
            "bo": bo_,
            "wg": np.ascontiguousarray(Wg_f[:, fs]),
            "wu": np.ascontiguousarray(Wu_f[:, fs]),
            "wd": np.ascontiguousarray(Wdown[fs, :]),
            "onesd": onesd_host,
            "maskt": maskt_host,
        })

    res = run_bass_kernel_spmd(nc, in_maps, core_ids=list(range(NCORES)),
                               **_CACHE.get("run_kwargs", {}))
    _CACHE["last_res"] = res

    out = np.empty((B, T, D), dtype=np.float32)
    for b in range(B):
        acc = res.results[b * TPG]["outT"].astype(np.float32)
        for g in range(1, TPG):
            acc = acc + res.results[b * TPG + g]["outT"]
        out[b] = acc.T
    return out

